# revision 1
# baseline (speedup 1.0000x reference)
"""Trainium2 Bass kernel for the fused einsum/groupconv/bmm module.

Math (per image n, C=256, H=W=56, HW=3136):
  t1[c,e] = sum_s X[c,s] P[s,e]
  t3      = groupconv3x3(x[n], conv_w, groups=2)
  t4      = p4 * t1;  t5[a] = sum_b t4[a,b] p5[b]
  out     = (t4 @ t3)/16 + broadcast((t5 @ X)/16)

Design (8 cores, 4 images each; ~159us PE busy / ~173us total per core
in the cost-model timeline sim, ~92% PE-array utilization):
  - Everything HBM-resident is bf16 (tolerance 2e-2 >> measured 4.4e-3);
    output bf16 on device, converted to f32 on host.
  - Every DMA is contiguous; the padded 58x58 conv layout is built
    ON-CHIP (persistent pad-zeroed tiles, interior rewritten per image
    by GpSimd).  No strided/tiny-descriptor DMAs anywhere.
  - conv: 9 shifted matmuls per group into PSUM, rhs as a 2-free-dim AP
    [8 rows x 56 cols, row stride 58] so only the 448 real output
    columns are streamed (no dead pad-column cycles); the whole chunk
    pipeline (cv, t3, t6, t7, ob) is uniformly 448-compact.
  - X^T for t1 via PE transposes straight from the UNPADDED x rows
    (25 chunks of 128); t1 contracts over unpadded s.
  - t7: two M=1 matmuls per chunk on unpadded x + GpSimd partition
    broadcast; the PSUM->output copies are contiguous DVE adds (t6+t7).
  - Startup is DMA-bandwidth-bound (~12us for x+wt+p1), so image 0 runs
    [tr cb0 -> ALL of conv g0 (needs only x-cb0+wt) -> tr cb1 -> t1 ->
    chunk loop], hiding the whole DMA window; small constants ride the
    Act HWDGE queue; x/p1 loads are split so consumers start early.
  - PSUM->SBUF copies split across DVE and Act; stores staged in 3
    pieces per image so only ~0.2MB remains after the final add.
"""

import sys

sys.path.insert(0, "/opt/trn_rl_repo")

import numpy as np

N, C, H, W = 32, 256, 56, 56
HW = H * W            # 3136
PH = H + 2            # 58
PHW = PH * PH         # 3364
XLEN = PHW + 3        # guard elem each end + 1 for the compact-conv AP view
NCORES = 8
NPER = N // NCORES    # 4 images per core
CHP = 8 * PH          # padded chunk: 8 padded rows = 464
NCHUNK = 7            # row starts 1,9,...,49 cover out rows 1..56
KP = 128              # transpose chunk (contiguous in UNPADDED x)
KT = 25               # ceil(HW/KP); t1 contraction over unpadded s
SLAST = HW - 24 * KP  # 64: width of the last (partial) transpose chunk
INV = 1.0 / 16.0      # 1/sqrt(C)


def build_body(tc, outs, ins):
    import concourse.mybir as mybir

    nc = tc.nc
    f32 = mybir.dt.float32
    bf16 = mybir.dt.bfloat16

    x_d = ins["x"]          # (NPER, C, HW)      bf16
    p1_d = ins["p1"]        # (KP, KT*C)         bf16 (pad rows zero)
    wt_d = ins["wt"]        # (128, 2*9*128)     bf16 [i, (g,t,o)]
    p4_d = ins["p4s"]       # (128, 2*C)         f32  [b, (bb,a)], pre-scaled
    p5_d = ins["p5"]        # (128, 2)           bf16 [b, bb]
    out_d = outs["out"]     # (NPER, C, HW)      bf16

    with (
        tc.tile_pool(name="const", bufs=1) as constp,
        tc.tile_pool(name="xrawp", bufs=2) as xrawp,
        tc.tile_pool(name="xtp", bufs=1) as xtp,
        tc.tile_pool(name="t3p", bufs=4) as t3p,
        tc.tile_pool(name="t3p0", bufs=7) as t3p0,
        tc.tile_pool(name="svp", bufs=2) as svp,
        tc.tile_pool(name="outp", bufs=2) as outp,
        tc.tile_pool(name="ps_tr", bufs=3, space="PSUM") as ps_tr,
        tc.tile_pool(name="ps_acc", bufs=1, space="PSUM") as ps_acc,
        tc.tile_pool(name="ps_cv", bufs=2, space="PSUM") as ps_cv,
        tc.tile_pool(name="ps_t6", bufs=2, space="PSUM") as ps_t6,
    ):
        # ---- startup-ordered DMAs: identity (tiny, unblocks transposes),
        # image-0 x per c-block, p1 halves (t1 needs the first half right
        # after the transposes), then the rest ----
        # Startup is DMA-bandwidth-bound (~12us to land x+wt+p1), so the
        # image-0 PE program is reordered to [tr cb0 -> conv g0 -> tr cb1
        # -> t1 -> rest]: conv g0 is ~12us of PE work that needs only
        # x-cb0 and wt, covering the whole DMA window.
        xraw0 = xrawp.tile([128, 2 * HW], bf16, tag="xraw", name="xraw")
        nc.sync.dma_start(
            out=xraw0[:, 0 : 12 * KP], in_=x_d[0, 0:128, 0 : 12 * KP]
        )
        nc.sync.dma_start(
            out=xraw0[:, 12 * KP : HW], in_=x_d[0, 0:128, 12 * KP : HW]
        )
        # small constants ride the Activation HWDGE queue so their
        # dispatch doesn't serialize behind the big SP-queue loads
        ident = constp.tile([128, 128], bf16, name="ident")
        nc.scalar.dma_start(out=ident[:, :], in_=ins["ident"])
        p4_sb = constp.tile([128, 2 * C], f32, name="p4_sb")
        nc.scalar.dma_start(out=p4_sb[:, :], in_=p4_d)
        p5_sb = constp.tile([128, 2], bf16, name="p5_sb")
        nc.scalar.dma_start(out=p5_sb[:, :], in_=p5_d)

        wt_sb = constp.tile([128, 2 * 9 * 128], bf16, name="wt_sb")
        nc.sync.dma_start(out=wt_sb[:, :], in_=wt_d)
        p1_sb = constp.tile([KP, KT * C], bf16, name="p1_sb")
        nc.sync.dma_start(
            out=p1_sb[:, 0 : 13 * C], in_=p1_d[:, 0 : 13 * C]
        )
        nc.sync.dma_start(
            out=xraw0[:, HW : 2 * HW], in_=x_d[0, 128:256]
        )
        nc.sync.dma_start(
            out=p1_sb[:, 13 * C : KT * C], in_=p1_d[:, 13 * C : KT * C]
        )

        # persistent padded-x tiles: pad positions zeroed once (head, the
        # 2-wide row seams, tail); per image only interior cols are
        # rewritten, pads stay zero.  Parity-0 pads are zeroed on DVE/Act
        # (fast, unblocks image 0); parity-1 on GpSimd (hidden).
        xpads = [
            [
                constp.tile([128, XLEN], bf16, name=f"xpad{q}{cb}")
                for cb in range(2)
            ]
            for q in range(2)
        ]
        for q in range(2):
            for cb in range(2):
                xp = xpads[q][cb]
                eng = nc.vector if q == 0 else nc.gpsimd
                eng.memset(xp[:, 0:60], 0.0)
                eng.memset(
                    xp[:, 116 : 116 + 55 * PH].rearrange(
                        "p (r w) -> p r w", w=PH
                    )[:, :, 0:2],
                    0.0,
                )
                eng.memset(xp[:, 3306:XLEN], 0.0)

        for n in range(NPER):
            q = n % 2
            xp0, xp1 = xpads[q]

            # ---- load x contiguously, both c-blocks in one DMA ----
            if n == 0:
                xraw = xraw0
            else:
                xraw = xrawp.tile([128, 2 * HW], bf16, tag="xraw", name="xraw")
                nc.sync.dma_start(
                    out=xraw.rearrange("p (cb s) -> p cb s", cb=2),
                    in_=x_d[n].rearrange("(cb p) s -> p cb s", cb=2),
                )
            # ---- build padded interior on-chip.  Image 0 on DVE/Act (on
            # the critical path at startup); later images on GpSimd (off
            # the critical engines, overlaps with prev image's work) ----
            for cb, xp in ((0, xp0), (1, xp1)):
                dst = xp[:, 60 : 60 + 56 * PH].rearrange(
                    "p (r w) -> p r w", w=PH
                )[:, :, 0:56]
                src = xraw[:, cb * HW : (cb + 1) * HW].rearrange(
                    "p (r w) -> p r w", w=W
                )
                if n == 0 and cb == 0:
                    # image-0 conv g0 runs early: this pad is on the
                    # critical path, so build it on the (idle) DVE
                    nc.vector.tensor_copy(dst, src)
                else:
                    nc.gpsimd.tensor_copy(dst, src)

            # ---- X^T via PE transposes straight from the UNPADDED xraw
            # (t1 contracts over unpadded s, so this never waits on the
            # pad-build).  Both c-blocks of one k share a PSUM buf -> one
            # 256-wide copy.  Image 0 runs cb-major so cb0's transposes
            # overlap cb1's load. ----
            xt = xtp.tile([KP, KT * C], bf16, tag="xt", name="xt")

            def tr_chunk(k, cb, trp, pcol):
                w = KP if k < KT - 1 else SLAST
                nc.tensor.transpose(
                    trp[0:w, pcol : pcol + 128],
                    xraw[:, cb * HW + k * KP : cb * HW + k * KP + w],
                    ident[:, :],
                )

            def conv_g(g, xp, c, pool, on_dve):
                r0 = 1 + 8 * c
                # stream only the 448 real output columns per tap: the rhs
                # is a 2-free-dim AP [8 rows x 56 cols] with row stride 58,
                # skipping the 16 dead pad columns of the 464-wide window
                cv = ps_cv.tile([128, 448], f32, tag="cv", name="cv")
                for tap in range(9):
                    kh, kw = tap // 3, tap % 3
                    foff = (r0 + kh - 1) * PH + kw
                    nc.tensor.matmul(
                        cv[:, :],
                        wt_sb[
                            :, (g * 9 + tap) * 128 : (g * 9 + tap) * 128 + 128
                        ],
                        xp[:, foff + 1 : foff + 1 + 8 * PH].rearrange(
                            "p (r w) -> p r w", w=PH
                        )[:, :, 0:56],
                        start=(tap == 0),
                        stop=(tap == 8),
                    )
                t3g = pool.tile([128, 448], bf16, tag=f"t3{g}", name="t3g")
                if on_dve:
                    nc.vector.tensor_copy(t3g[:, :], cv[:, :])
                else:
                    nc.scalar.copy(t3g[:, :], cv[:, :])
                return t3g

            t3g0s = []
            if n == 0:
                # cb-major, pairing consecutive k so each 256-wide PSUM
                # buf drains with one copy (copies keep up with the PE).
                # Between cb0 and cb1 run ALL of conv g0 (~12us of PE work
                # needing only x-cb0 + wt) to cover the DMA-bound startup.
                xtv = xt.rearrange("p (k c) -> p k c", c=C)
                for cb in range(2):
                    for k0 in range(0, KT - 1, 2):
                        trp = ps_tr.tile([KP, 256], bf16, tag="tr", name="trp")
                        tr_chunk(k0, cb, trp, 0)
                        tr_chunk(k0 + 1, cb, trp, 128)
                        dst = xtv[:, k0 : k0 + 2, cb * 128 : cb * 128 + 128]
                        if k0 % 4 == 0:
                            nc.vector.tensor_copy(
                                dst, trp.rearrange("p (k c) -> p k c", c=128)
                            )
                        else:
                            nc.scalar.copy(
                                dst, trp.rearrange("p (k c) -> p k c", c=128)
                            )
                    trp = ps_tr.tile([KP, 256], bf16, tag="tr", name="trp")
                    tr_chunk(KT - 1, cb, trp, 0)
                    dst = xt[
                        0:SLAST,
                        (KT - 1) * C + cb * 128 : (KT - 1) * C + cb * 128 + 128,
                    ]
                    nc.vector.tensor_copy(dst, trp[0:SLAST, 0:128])
                    if cb == 0:
                        for c in range(NCHUNK):
                            t3g0s.append(
                                conv_g(0, xp0, c, t3p0, c % 2 == 0)
                            )
            else:
                for k in range(KT):
                    trp = ps_tr.tile([KP, 256], bf16, tag="tr", name="trp")
                    w = KP if k < KT - 1 else SLAST
                    for cb in range(2):
                        tr_chunk(k, cb, trp, cb * 128)
                    dst = xt[0:w, k * C : (k + 1) * C]
                    if k % 2 == 0:
                        nc.vector.tensor_copy(dst, trp[0:w, :])
                    else:
                        nc.scalar.copy(dst, trp[0:w, :])

            # ---- t1T = P^T @ X^T;  t4T = p4s * t1T  (b-part, a-free) ----
            t4T = svp.tile([128, 2 * C], bf16, tag="t4", name="t4T")
            for eb in range(2):
                t1ps = ps_acc.tile([128, C], f32, tag="acc", name="t1ps")
                for k in range(KT):
                    w = KP if k < KT - 1 else SLAST
                    nc.tensor.matmul(
                        t1ps[:, :],
                        p1_sb[0:w, k * C + eb * 128 : k * C + eb * 128 + 128],
                        xt[0:w, k * C : (k + 1) * C],
                        start=(k == 0),
                        stop=(k == KT - 1),
                    )
                nc.vector.tensor_mul(
                    t4T[:, eb * C : (eb + 1) * C],
                    t1ps[:, :],
                    p4_sb[:, eb * C : (eb + 1) * C],
                )

            # ---- t5[a] = sum_b t4T[b,a] p5[b]; broadcast to lhsT ----
            t5ps = ps_acc.tile([128, 2], f32, tag="acc", name="t5ps")
            for ab in range(2):
                for bb in range(2):
                    nc.tensor.matmul(
                        t5ps[:, ab : ab + 1],
                        t4T[:, bb * C + ab * 128 : bb * C + ab * 128 + 128],
                        p5_sb[:, bb : bb + 1],
                        start=(bb == 0),
                        stop=(bb == 1),
                    )
            t5col = svp.tile([128, 2], bf16, tag="t5", name="t5col")
            nc.scalar.copy(t5col[:, :], t5ps[:, :])

            # ---- chunk loop: conv then t6(+t7), accumulate into obig ----
            ob = outp.tile([128, 2 * HW], bf16, tag="ob", name="ob")
            for c in range(NCHUNK):
                r0 = 1 + 8 * c          # padded row of chunk start
                f0 = 1 + r0 * PH        # flat start of chunk in xpad

                # t7 row for this chunk straight from the UNPADDED xraw
                # (8 rows = 448 contiguous cols): 2 K=128,M=1 matmuls, then
                # a GpSimd partition-broadcast so the output copies can add
                # it elementwise (keeps the broadcast off the PE).
                t7ps = ps_acc.tile([1, 448], f32, tag="acc", name="t7ps")
                for cb in range(2):
                    nc.tensor.matmul(
                        t7ps[:, :],
                        t5col[:, cb : cb + 1],
                        xraw[:, cb * HW + c * 448 : cb * HW + (c + 1) * 448],
                        start=(cb == 0),
                        stop=(cb == 1),
                    )
                # GpSimd can't read PSUM: bounce the row through SBUF first
                t7row = svp.tile([1, 448], f32, tag="t7r", name="t7row")
                nc.scalar.copy(t7row[:, :], t7ps[:, :])
                t7b = svp.tile([128, 448], f32, tag="t7b", name="t7b")
                nc.gpsimd.partition_broadcast(t7b[:, :], t7row[:, :])

                if n == 0:
                    t3c = [t3g0s[c], conv_g(1, xp1, c, t3p, False)]
                else:
                    t3c = [
                        conv_g(0, xp0, c, t3p, False),
                        conv_g(1, xp1, c, t3p, False),
                    ]

                for ab in range(2):
                    t6ps = ps_t6.tile([128, 448], f32, tag="t6", name="t6ps")
                    for bb in range(2):
                        nc.tensor.matmul(
                            t6ps[:, :],
                            t4T[:, bb * C + ab * 128 : bb * C + ab * 128 + 128],
                            t3c[bb][:, :],
                            start=(bb == 0),
                            stop=(bb == 1),
                        )
                    # everything is compact now: contiguous add of the
                    # broadcast t7 row while copying PSUM -> output tile
                    nc.vector.tensor_add(
                        ob[:, ab * HW + c * 448 : ab * HW + (c + 1) * 448],
                        t6ps[:, :],
                        t7b[:, :],
                    )

                # stage the store so only the last chunk's ~0.2MB of store
                # remains after the final add
                if c in (3, 5):
                    lo = 0 if c == 3 else 4 * 448
                    hi = (c + 1) * 448
                    nc.sync.dma_start(
                        out=out_d[n, :, lo:hi].rearrange(
                            "(ab p) s -> p ab s", ab=2
                        ),
                        in_=ob.rearrange("p (ab s) -> p ab s", ab=2)[
                            :, :, lo:hi
                        ],
                    )
            nc.sync.dma_start(
                out=out_d[n, :, 6 * 448 : HW].rearrange(
                    "(ab p) s -> p ab s", ab=2
                ),
                in_=ob.rearrange("p (ab s) -> p ab s", ab=2)[:, :, 6 * 448 : HW],
            )


_CACHE = {}


def _get_nc():
    if "nc" in _CACHE:
        return _CACHE["nc"]
    import concourse.bacc as bacc
    import concourse.mybir as mybir
    import concourse.tile as tile

    f32 = mybir.dt.float32
    bf16 = mybir.dt.bfloat16
    nc = bacc.Bacc(
        "TRN2", target_bir_lowering=False, debug=False, num_devices=NCORES
    )
    ins = {
        "x": nc.dram_tensor("x", (NPER, C, HW), bf16, kind="ExternalInput").ap(),
        "p1": nc.dram_tensor("p1", (KP, KT * C), bf16, kind="ExternalInput").ap(),
        "wt": nc.dram_tensor("wt", (128, 2 * 9 * 128), bf16, kind="ExternalInput").ap(),
        "p4s": nc.dram_tensor("p4s", (128, 2 * C), f32, kind="ExternalInput").ap(),
        "p5": nc.dram_tensor("p5", (128, 2), bf16, kind="ExternalInput").ap(),
        "ident": nc.dram_tensor("ident", (128, 128), bf16, kind="ExternalInput").ap(),
    }
    outs = {
        "out": nc.dram_tensor("out", (NPER, C, HW), bf16, kind="ExternalOutput").ap(),
    }
    with tile.TileContext(nc) as tc:
        build_body(tc, outs, ins)
    nc.compile()
    _CACHE["nc"] = nc
    return nc


def host_prep(inputs):
    """Split full inputs into per-core in_maps (host-side relayout + bf16)."""
    import ml_dtypes

    bf = ml_dtypes.bfloat16
    x = np.asarray(inputs["x"], dtype=np.float32).reshape(N, C, HW).astype(bf)
    p1p = np.zeros((KT * KP, C), dtype=np.float32)
    p1p[:HW] = np.asarray(inputs["p1_w"], dtype=np.float32)[..., 0].reshape(
        HW, C
    )
    p1h = np.ascontiguousarray(
        p1p.reshape(KT, KP, C).transpose(1, 0, 2).reshape(KP, KT * C)
    ).astype(bf)
    wt = np.ascontiguousarray(
        np.asarray(inputs["conv_w"], dtype=np.float32)
        .reshape(2, 128, 128, 9)
        .transpose(3, 0, 1, 2)       # t, g, o, i -> want [i, (g,t,o)]
        .transpose(3, 1, 0, 2)       # i, g, t, o
        .reshape(128, 2 * 9 * 128)
    ).astype(bf)
    p4s = np.ascontiguousarray(
        (np.asarray(inputs["p4_w"], dtype=np.float32)[0].T * INV)
        .reshape(2, 128, C)
        .transpose(1, 0, 2)
        .reshape(128, 2 * C)
    )
    identm = np.eye(128, dtype=np.float32).astype(bf)
    p5 = np.ascontiguousarray(
        np.asarray(inputs["p5_w"], dtype=np.float32).reshape(2, 128).T
    ).astype(bf)
    xs = x.reshape(NCORES, NPER, C, HW)
    return [
        {
            "x": np.ascontiguousarray(xs[i]), "p1": p1h, "wt": wt,
            "p4s": p4s, "p5": p5, "ident": identm,
        }
        for i in range(NCORES)
    ]


def kernel(**inputs):
    from concourse.bass_utils import run_bass_kernel_spmd

    nc = _get_nc()
    in_maps = host_prep(inputs)
    res = run_bass_kernel_spmd(nc, in_maps, core_ids=list(range(NCORES)))
    out = np.concatenate([res.results[i]["out"] for i in range(NCORES)], axis=0)
    return out.astype(np.float32).reshape(N, C, H, W)



# revision 6
# speedup vs baseline: 1.1815x; 1.1815x over previous
"""Trainium2 Bass kernel for the fused einsum/groupconv/bmm module.

Math (per image n, C=256, H=W=56, HW=3136):
  t1[c,e] = sum_s X[c,s] P[s,e]
  t3      = groupconv3x3(x[n], conv_w, groups=2)
  t4      = p4 * t1;  t5[a] = sum_b t4[a,b] p5[b]
  out     = (t4 @ t3)/16 + broadcast((t5 @ X)/16)

Design (8 cores, 4 images each; ~159us PE busy / ~173us total per core
in the cost-model timeline sim, ~92% PE-array utilization):
  - Everything HBM-resident is bf16 (tolerance 2e-2 >> measured 4.4e-3);
    output bf16 on device, converted to f32 on host.
  - Every DMA is contiguous; the padded 58x58 conv layout is built
    ON-CHIP (persistent pad-zeroed tiles, interior rewritten per image
    by GpSimd).  No strided/tiny-descriptor DMAs anywhere.
  - conv: 9 shifted matmuls per group into PSUM, rhs as a 2-free-dim AP
    [8 rows x 56 cols, row stride 58] so only the 448 real output
    columns are streamed (no dead pad-column cycles); the whole chunk
    pipeline (cv, t3, t6, t7, ob) is uniformly 448-compact.
  - X^T for t1 via PE transposes straight from the UNPADDED x rows
    (25 chunks of 128); t1 contracts over unpadded s.
  - t7: two M=1 matmuls per chunk on unpadded x + GpSimd partition
    broadcast; the PSUM->output copies are contiguous DVE adds (t6+t7).
  - Startup is DMA-bandwidth-bound (~12us for x+wt+p1), so image 0 runs
    [tr cb0 -> ALL of conv g0 (needs only x-cb0+wt) -> tr cb1 -> t1 ->
    chunk loop], hiding the whole DMA window; small constants ride the
    Act HWDGE queue; x/p1 loads are split so consumers start early.
  - PSUM->SBUF copies split across DVE and Act; stores staged in 3
    pieces per image so only ~0.2MB remains after the final add.
"""

import sys

sys.path.insert(0, "/opt/trn_rl_repo")

import numpy as np

N, C, H, W = 32, 256, 56, 56
HW = H * W            # 3136
PH = H + 2            # 58
PHW = PH * PH         # 3364
XLEN = PHW + 3        # guard elem each end + 1 for the compact-conv AP view
NCORES = 8
NPER = N // NCORES    # 4 images per core
CHP = 8 * PH          # padded chunk: 8 padded rows = 464
NCHUNK = 7            # row starts 1,9,...,49 cover out rows 1..56
KP = 128              # transpose chunk (contiguous in UNPADDED x)
KT = 25               # ceil(HW/KP); t1 contraction over unpadded s
SLAST = HW - 24 * KP  # 64: width of the last (partial) transpose chunk
INV = 1.0 / 16.0      # 1/sqrt(C)


def build_body(tc, outs, ins):
    import concourse.mybir as mybir

    nc = tc.nc
    f32 = mybir.dt.float32
    bf16 = mybir.dt.bfloat16

    x_d = ins["x"]          # (NPER, C, HW)      bf16
    p1_d = ins["p1"]        # (KP, KT*C)         bf16 (pad rows zero)
    wt_d = ins["wt"]        # (128, 2*9*128)     bf16 [i, (g,t,o)]
    p4_d = ins["p4s"]       # (128, 2*C)         f32  [b, (bb,a)], pre-scaled
    p5_d = ins["p5"]        # (128, 2)           bf16 [b, bb]
    out_d = outs["out"]     # (NPER, C, HW)      bf16

    with (
        tc.tile_pool(name="const", bufs=1) as constp,
        tc.tile_pool(name="xrawp", bufs=2) as xrawp,
        tc.tile_pool(name="xtp", bufs=1) as xtp,
        tc.tile_pool(name="t3p", bufs=4) as t3p,
        tc.tile_pool(name="t3p0", bufs=7) as t3p0,
        tc.tile_pool(name="svp", bufs=2) as svp,
        tc.tile_pool(name="outp", bufs=2) as outp,
        tc.tile_pool(name="ps_tr", bufs=3, space="PSUM") as ps_tr,
        tc.tile_pool(name="ps_acc", bufs=1, space="PSUM") as ps_acc,
        tc.tile_pool(name="ps_cv", bufs=2, space="PSUM") as ps_cv,
        tc.tile_pool(name="ps_t6", bufs=2, space="PSUM") as ps_t6,
    ):
        # ---- startup-ordered DMAs: identity (tiny, unblocks transposes),
        # image-0 x per c-block, p1 halves (t1 needs the first half right
        # after the transposes), then the rest ----
        # Startup is DMA-bandwidth-bound (~12us to land x+wt+p1), so the
        # image-0 PE program is reordered to [tr cb0 -> conv g0 -> tr cb1
        # -> t1 -> rest]: conv g0 is ~12us of PE work that needs only
        # x-cb0 and wt, covering the whole DMA window.
        xraw0 = xrawp.tile([128, 2 * HW], bf16, tag="xraw", name="xraw")
        nc.sync.dma_start(
            out=xraw0[:, 0 : 12 * KP], in_=x_d[0, 0:128, 0 : 12 * KP]
        )
        nc.sync.dma_start(
            out=xraw0[:, 12 * KP : HW], in_=x_d[0, 0:128, 12 * KP : HW]
        )
        # small constants ride the Activation HWDGE queue so their
        # dispatch doesn't serialize behind the big SP-queue loads
        ident = constp.tile([128, 128], bf16, name="ident")
        nc.scalar.dma_start(out=ident[:, :], in_=ins["ident"])
        p4_sb = constp.tile([128, 2 * C], f32, name="p4_sb")
        nc.scalar.dma_start(out=p4_sb[:, :], in_=p4_d)
        p5_sb = constp.tile([128, 2], bf16, name="p5_sb")
        nc.scalar.dma_start(out=p5_sb[:, :], in_=p5_d)
        ones_sb = constp.tile([128, 128], bf16, name="ones_sb")
        nc.vector.memset(ones_sb[:, :], 1.0)

        wt_sb = constp.tile([128, 2 * 9 * 128], bf16, name="wt_sb")
        nc.sync.dma_start(out=wt_sb[:, :], in_=wt_d)
        p1_sb = constp.tile([KP, KT * C], bf16, name="p1_sb")
        nc.sync.dma_start(
            out=p1_sb[:, 0 : 13 * C], in_=p1_d[:, 0 : 13 * C]
        )
        nc.sync.dma_start(
            out=xraw0[:, HW : 2 * HW], in_=x_d[0, 128:256]
        )
        nc.sync.dma_start(
            out=p1_sb[:, 13 * C : KT * C], in_=p1_d[:, 13 * C : KT * C]
        )

        # persistent padded-x tiles: pad positions zeroed once (head, the
        # 2-wide row seams, tail); per image only interior cols are
        # rewritten, pads stay zero.  Parity-0 pads are zeroed on DVE/Act
        # (fast, unblocks image 0); parity-1 on GpSimd (hidden).
        xpads = [
            [
                constp.tile([128, XLEN], bf16, name=f"xpad{q}{cb}")
                for cb in range(2)
            ]
            for q in range(2)
        ]
        for q in range(2):
            for cb in range(2):
                xp = xpads[q][cb]
                eng = nc.vector if q == 0 else nc.gpsimd
                eng.memset(xp[:, 0:60], 0.0)
                eng.memset(
                    xp[:, 116 : 116 + 55 * PH].rearrange(
                        "p (r w) -> p r w", w=PH
                    )[:, :, 0:2],
                    0.0,
                )
                eng.memset(xp[:, 3306:XLEN], 0.0)

        for n in range(NPER):
            q = n % 2
            xp0, xp1 = xpads[q]

            # ---- load x contiguously, both c-blocks in one DMA ----
            if n == 0:
                xraw = xraw0
            else:
                xraw = xrawp.tile([128, 2 * HW], bf16, tag="xraw", name="xraw")
                nc.sync.dma_start(
                    out=xraw.rearrange("p (cb s) -> p cb s", cb=2),
                    in_=x_d[n].rearrange("(cb p) s -> p cb s", cb=2),
                )
            # ---- build padded interior on-chip, on DVE (bf16 SBUF->SBUF
            # copies run 4x there, ~1us; HW GpSimd took ~10.6us each and
            # serialized the whole inter-image pipeline) ----
            for cb, xp in ((0, xp0), (1, xp1)):
                dst = xp[:, 60 : 60 + 56 * PH].rearrange(
                    "p (r w) -> p r w", w=PH
                )[:, :, 0:56]
                src = xraw[:, cb * HW : (cb + 1) * HW].rearrange(
                    "p (r w) -> p r w", w=W
                )
                nc.vector.tensor_copy(dst, src)

            # ---- X^T via PE transposes straight from the UNPADDED xraw
            # (t1 contracts over unpadded s, so this never waits on the
            # pad-build).  Both c-blocks of one k share a PSUM buf -> one
            # 256-wide copy.  Image 0 runs cb-major so cb0's transposes
            # overlap cb1's load. ----
            xt = xtp.tile([KP, KT * C], bf16, tag="xt", name="xt")

            def tr_chunk(k, cb, trp, pcol):
                w = KP if k < KT - 1 else SLAST
                nc.tensor.transpose(
                    trp[0:w, pcol : pcol + 128],
                    xraw[:, cb * HW + k * KP : cb * HW + k * KP + w],
                    ident[:, :],
                )

            def conv_g(g, xp, c, pool, on_dve):
                r0 = 1 + 8 * c
                # stream only the 448 real output columns per tap: the rhs
                # is a 2-free-dim AP [8 rows x 56 cols] with row stride 58,
                # skipping the 16 dead pad columns of the 464-wide window
                cv = ps_cv.tile([128, 448], f32, tag="cv", name="cv")
                for tap in range(9):
                    kh, kw = tap // 3, tap % 3
                    foff = (r0 + kh - 1) * PH + kw
                    nc.tensor.matmul(
                        cv[:, :],
                        wt_sb[
                            :, (g * 9 + tap) * 128 : (g * 9 + tap) * 128 + 128
                        ],
                        xp[:, foff + 1 : foff + 1 + 8 * PH].rearrange(
                            "p (r w) -> p r w", w=PH
                        )[:, :, 0:56],
                        start=(tap == 0),
                        stop=(tap == 8),
                    )
                t3g = pool.tile([128, 448], bf16, tag=f"t3{g}", name="t3g")
                if on_dve:
                    nc.vector.tensor_copy(t3g[:, :], cv[:, :])
                else:
                    nc.scalar.copy(t3g[:, :], cv[:, :])
                return t3g

            t3g0s = []
            if n == 0:
                # cb-major, pairing consecutive k so each 256-wide PSUM
                # buf drains with one copy (copies keep up with the PE).
                # Between cb0 and cb1 run ALL of conv g0 (~12us of PE work
                # needing only x-cb0 + wt) to cover the DMA-bound startup.
                xtv = xt.rearrange("p (k c) -> p k c", c=C)
                for cb in range(2):
                    for k0 in range(0, KT - 1, 2):
                        trp = ps_tr.tile([KP, 256], bf16, tag="tr", name="trp")
                        tr_chunk(k0, cb, trp, 0)
                        tr_chunk(k0 + 1, cb, trp, 128)
                        dst = xtv[:, k0 : k0 + 2, cb * 128 : cb * 128 + 128]
                        if k0 % 4 == 0:
                            nc.vector.tensor_copy(
                                dst, trp.rearrange("p (k c) -> p k c", c=128)
                            )
                        else:
                            nc.scalar.copy(
                                dst, trp.rearrange("p (k c) -> p k c", c=128)
                            )
                    trp = ps_tr.tile([KP, 256], bf16, tag="tr", name="trp")
                    tr_chunk(KT - 1, cb, trp, 0)
                    dst = xt[
                        0:SLAST,
                        (KT - 1) * C + cb * 128 : (KT - 1) * C + cb * 128 + 128,
                    ]
                    nc.vector.tensor_copy(dst, trp[0:SLAST, 0:128])
                    if cb == 0:
                        for c in range(NCHUNK):
                            t3g0s.append(
                                conv_g(0, xp0, c, t3p0, c % 2 == 0)
                            )
            else:
                for k in range(KT):
                    trp = ps_tr.tile([KP, 256], bf16, tag="tr", name="trp")
                    w = KP if k < KT - 1 else SLAST
                    for cb in range(2):
                        tr_chunk(k, cb, trp, cb * 128)
                    dst = xt[0:w, k * C : (k + 1) * C]
                    if k % 2 == 0:
                        nc.vector.tensor_copy(dst, trp[0:w, :])
                    else:
                        nc.scalar.copy(dst, trp[0:w, :])

            # ---- t1T = P^T @ X^T;  t4T = p4s * t1T  (b-part, a-free) ----
            t4T = svp.tile([128, 2 * C], bf16, tag="t4", name="t4T")
            for eb in range(2):
                t1ps = ps_acc.tile([128, C], f32, tag="acc", name="t1ps")
                for k in range(KT):
                    w = KP if k < KT - 1 else SLAST
                    nc.tensor.matmul(
                        t1ps[:, :],
                        p1_sb[0:w, k * C + eb * 128 : k * C + eb * 128 + 128],
                        xt[0:w, k * C : (k + 1) * C],
                        start=(k == 0),
                        stop=(k == KT - 1),
                    )
                nc.vector.tensor_mul(
                    t4T[:, eb * C : (eb + 1) * C],
                    t1ps[:, :],
                    p4_sb[:, eb * C : (eb + 1) * C],
                )

            # ---- t5[a] = sum_b t4T[b,a] p5[b]; broadcast to lhsT ----
            t5ps = ps_acc.tile([128, 2], f32, tag="acc", name="t5ps")
            for ab in range(2):
                for bb in range(2):
                    nc.tensor.matmul(
                        t5ps[:, ab : ab + 1],
                        t4T[:, bb * C + ab * 128 : bb * C + ab * 128 + 128],
                        p5_sb[:, bb : bb + 1],
                        start=(bb == 0),
                        stop=(bb == 1),
                    )
            t5col = svp.tile([128, 2], f32, tag="t5", name="t5col")
            nc.scalar.copy(t5col[:, :], t5ps[:, :])
            # t5 broadcast as a stationary matrix: t5b[cb][b, a] = t5[cb*128+b]
            # for every a.  Lets the t7 row-broadcast ride the PE as two
            # extra accumulating matmuls per (chunk, ab) instead of the
            # GpSimd partition_broadcast + PSUM bounce chain.
            t5b = svp.tile([128, 2 * 128], bf16, tag="t5b", name="t5b")
            for cb in range(2):
                nc.vector.tensor_scalar_mul(
                    t5b[:, cb * 128 : (cb + 1) * 128],
                    ones_sb[:, :],
                    t5col[:, cb : cb + 1],
                )

            # ---- chunk loop: conv then t6(+t7), accumulate into obig ----
            ob = outp.tile([128, 2 * HW], bf16, tag="ob", name="ob")
            for c in range(NCHUNK):
                if n == 0:
                    t3c = [t3g0s[c], conv_g(1, xp1, c, t3p, False)]
                else:
                    t3c = [
                        conv_g(0, xp0, c, t3p, False),
                        conv_g(1, xp1, c, t3p, False),
                    ]

                for ab in range(2):
                    # t6ps = sum_bb t4T.T @ t3  +  sum_cb t5b.T @ x  (the
                    # second pair IS the broadcast t7 row: t5b columns are
                    # all equal, so every output partition gets t7[s])
                    t6ps = ps_t6.tile([128, 448], f32, tag="t6", name="t6ps")
                    for bb in range(2):
                        nc.tensor.matmul(
                            t6ps[:, :],
                            t4T[:, bb * C + ab * 128 : bb * C + ab * 128 + 128],
                            t3c[bb][:, :],
                            start=(bb == 0),
                            stop=False,
                        )
                    for cb in range(2):
                        nc.tensor.matmul(
                            t6ps[:, :],
                            t5b[:, cb * 128 : (cb + 1) * 128],
                            xraw[:, cb * HW + c * 448 : cb * HW + (c + 1) * 448],
                            start=False,
                            stop=(cb == 1),
                        )
                    nc.vector.tensor_copy(
                        ob[:, ab * HW + c * 448 : ab * HW + (c + 1) * 448],
                        t6ps[:, :],
                    )

                # stage the store so only the last chunk's ~0.2MB of store
                # remains after the final add
                if c in (3, 5):
                    lo = 0 if c == 3 else 4 * 448
                    hi = (c + 1) * 448
                    nc.sync.dma_start(
                        out=out_d[n, :, lo:hi].rearrange(
                            "(ab p) s -> p ab s", ab=2
                        ),
                        in_=ob.rearrange("p (ab s) -> p ab s", ab=2)[
                            :, :, lo:hi
                        ],
                    )
            nc.sync.dma_start(
                out=out_d[n, :, 6 * 448 : HW].rearrange(
                    "(ab p) s -> p ab s", ab=2
                ),
                in_=ob.rearrange("p (ab s) -> p ab s", ab=2)[:, :, 6 * 448 : HW],
            )


_CACHE = {}


def _get_nc():
    if "nc" in _CACHE:
        return _CACHE["nc"]
    import concourse.bacc as bacc
    import concourse.mybir as mybir
    import concourse.tile as tile

    f32 = mybir.dt.float32
    bf16 = mybir.dt.bfloat16
    nc = bacc.Bacc(
        "TRN2", target_bir_lowering=False, debug=False, num_devices=NCORES
    )
    ins = {
        "x": nc.dram_tensor("x", (NPER, C, HW), bf16, kind="ExternalInput").ap(),
        "p1": nc.dram_tensor("p1", (KP, KT * C), bf16, kind="ExternalInput").ap(),
        "wt": nc.dram_tensor("wt", (128, 2 * 9 * 128), bf16, kind="ExternalInput").ap(),
        "p4s": nc.dram_tensor("p4s", (128, 2 * C), f32, kind="ExternalInput").ap(),
        "p5": nc.dram_tensor("p5", (128, 2), bf16, kind="ExternalInput").ap(),
        "ident": nc.dram_tensor("ident", (128, 128), bf16, kind="ExternalInput").ap(),
    }
    outs = {
        "out": nc.dram_tensor("out", (NPER, C, HW), bf16, kind="ExternalOutput").ap(),
    }
    with tile.TileContext(nc) as tc:
        build_body(tc, outs, ins)
    nc.compile()
    _CACHE["nc"] = nc
    return nc


def host_prep(inputs):
    """Split full inputs into per-core in_maps (host-side relayout + bf16)."""
    import ml_dtypes

    bf = ml_dtypes.bfloat16
    x = np.asarray(inputs["x"], dtype=np.float32).reshape(N, C, HW).astype(bf)
    p1p = np.zeros((KT * KP, C), dtype=np.float32)
    p1p[:HW] = np.asarray(inputs["p1_w"], dtype=np.float32)[..., 0].reshape(
        HW, C
    )
    p1h = np.ascontiguousarray(
        p1p.reshape(KT, KP, C).transpose(1, 0, 2).reshape(KP, KT * C)
    ).astype(bf)
    wt = np.ascontiguousarray(
        np.asarray(inputs["conv_w"], dtype=np.float32)
        .reshape(2, 128, 128, 9)
        .transpose(3, 0, 1, 2)       # t, g, o, i -> want [i, (g,t,o)]
        .transpose(3, 1, 0, 2)       # i, g, t, o
        .reshape(128, 2 * 9 * 128)
    ).astype(bf)
    p4s = np.ascontiguousarray(
        (np.asarray(inputs["p4_w"], dtype=np.float32)[0].T * INV)
        .reshape(2, 128, C)
        .transpose(1, 0, 2)
        .reshape(128, 2 * C)
    )
    identm = np.eye(128, dtype=np.float32).astype(bf)
    p5 = np.ascontiguousarray(
        np.asarray(inputs["p5_w"], dtype=np.float32).reshape(2, 128).T
    ).astype(bf)
    xs = x.reshape(NCORES, NPER, C, HW)
    return [
        {
            "x": np.ascontiguousarray(xs[i]), "p1": p1h, "wt": wt,
            "p4s": p4s, "p5": p5, "ident": identm,
        }
        for i in range(NCORES)
    ]


def kernel(**inputs):
    from concourse.bass_utils import run_bass_kernel_spmd

    nc = _get_nc()
    in_maps = host_prep(inputs)
    res = run_bass_kernel_spmd(nc, in_maps, core_ids=list(range(NCORES)))
    out = np.concatenate([res.results[i]["out"] for i in range(NCORES)], axis=0)
    return out.astype(np.float32).reshape(N, C, H, W)



# revision 7
# speedup vs baseline: 1.1887x; 1.0061x over previous
"""Trainium2 Bass kernel for the fused einsum/groupconv/bmm module.

Math (per image n, C=256, H=W=56, HW=3136):
  t1[c,e] = sum_s X[c,s] P[s,e]
  t3      = groupconv3x3(x[n], conv_w, groups=2)
  t4      = p4 * t1;  t5[a] = sum_b t4[a,b] p5[b]
  out     = (t4 @ t3)/16 + broadcast((t5 @ X)/16)

Design (8 cores, 4 images each):
  - Everything HBM-resident is bf16 (tolerance 2e-2 >> measured 4.4e-3);
    output bf16 on device, converted to f32 on host.
  - X^T for t1 via the DMA-transpose xbar straight from DRAM (k=0..23;
    the w=64 tail chunk via a PE transpose) -- keeps ~8us/image of
    transposes + LDWEIGHTS off the PE, which is the bottleneck engine.
  - The padded 58x58 conv layout is built ON-CHIP: persistent pad-zeroed
    tiles, interior rewritten per image by DVE (bf16 SBUF copies run 4x
    there; GpSimd measured 10.6us per copy on HW and serialized the
    pipeline).
  - conv: 9 shifted matmuls per group into PSUM, rhs as a 2-free-dim AP
    [8 rows x 56 cols, row stride 58] so only the 448 real output
    columns are streamed; the whole chunk pipeline is 448-compact.
  - t7: per chunk, one broadcast PSUM tile via 2 matmuls with a
    stationary t5-broadcast matrix (t5b[b,a] = t5[b] for all a), bounced
    to SBUF once on ACT; the two PSUM->output drains are DVE adds.
  - Stores staged in 3 pieces per image so only ~0.2MB remains after the
    final add.
"""

import sys

sys.path.insert(0, "/opt/trn_rl_repo")

import numpy as np

N, C, H, W = 32, 256, 56, 56
HW = H * W            # 3136
PH = H + 2            # 58
PHW = PH * PH         # 3364
XLEN = PHW + 3        # guard elem each end + 1 for the compact-conv AP view
NCORES = 8
NPER = N // NCORES    # 4 images per core
CHP = 8 * PH          # padded chunk: 8 padded rows = 464
NCHUNK = 7            # row starts 1,9,...,49 cover out rows 1..56
KP = 128              # transpose chunk (contiguous in UNPADDED x)
KT = 25               # ceil(HW/KP); t1 contraction over unpadded s
KDMA = 24             # k-chunks transposed by the DMA xbar (full 128 wide)
SLAST = HW - 24 * KP  # 64: width of the last (partial) transpose chunk
INV = 1.0 / 16.0      # 1/sqrt(C)


def build_body(tc, outs, ins):
    import concourse.mybir as mybir

    nc = tc.nc
    f32 = mybir.dt.float32
    bf16 = mybir.dt.bfloat16

    x_d = ins["x"]          # (NPER, C, HW)      bf16
    p1_d = ins["p1"]        # (KP, KT*C)         bf16 (pad rows zero)
    wt_d = ins["wt"]        # (128, 2*9*128)     bf16 [i, (g,t,o)]
    p4_d = ins["p4s"]       # (128, 2*C)         f32  [b, (bb,a)], pre-scaled
    p5_d = ins["p5"]        # (128, 2)           bf16 [b, bb]
    out_d = outs["out"]     # (NPER, C, HW)      bf16

    with (
        tc.tile_pool(name="const", bufs=1) as constp,
        tc.tile_pool(name="xrawp", bufs=2) as xrawp,
        tc.tile_pool(name="xtp", bufs=2) as xtp,
        tc.tile_pool(name="t3p", bufs=4) as t3p,
        tc.tile_pool(name="svp", bufs=2) as svp,
        tc.tile_pool(name="outp", bufs=2) as outp,
        tc.tile_pool(name="ps_tail", bufs=1, space="PSUM") as ps_tail,
        tc.tile_pool(name="ps_acc", bufs=1, space="PSUM") as ps_acc,
        tc.tile_pool(name="ps_cv", bufs=3, space="PSUM") as ps_cv,
        tc.tile_pool(name="ps_t6", bufs=2, space="PSUM") as ps_t6,
        tc.tile_pool(name="ps_t7", bufs=1, space="PSUM") as ps_t7,
    ):
        # ---- startup DMAs.  The sync (SP) HWDGE ring carries the big x
        # loads + stores; the scalar (ACT) ring carries the constants and
        # all DMA-transposes (keeps xbar-mode transitions off the copy
        # ring). ----
        ident = constp.tile([128, 128], bf16, name="ident")
        nc.scalar.dma_start(out=ident[:, :], in_=ins["ident"])
        p4_sb = constp.tile([128, 2 * C], f32, name="p4_sb")
        nc.scalar.dma_start(out=p4_sb[:, :], in_=p4_d)
        p5_sb = constp.tile([128, 2], bf16, name="p5_sb")
        nc.scalar.dma_start(out=p5_sb[:, :], in_=p5_d)
        ones_sb = constp.tile([128, 128], bf16, name="ones_sb")
        nc.vector.memset(ones_sb[:, :], 1.0)

        p1_sb = constp.tile([KP, KT * C], bf16, name="p1_sb")
        nc.scalar.dma_start(
            out=p1_sb[:, 0 : 13 * C], in_=p1_d[:, 0 : 13 * C]
        )
        wt_sb = constp.tile([128, 2 * 9 * 128], bf16, name="wt_sb")
        nc.sync.dma_start(out=wt_sb[:, :], in_=wt_d)
        nc.scalar.dma_start(
            out=p1_sb[:, 13 * C : KT * C], in_=p1_d[:, 13 * C : KT * C]
        )

        # persistent padded-x tiles: pad positions zeroed once (head, the
        # 2-wide row seams, tail); per image only interior cols are
        # rewritten, pads stay zero.
        xpads = [
            [
                constp.tile([128, XLEN], bf16, name=f"xpad{q}{cb}")
                for cb in range(2)
            ]
            for q in range(2)
        ]
        for q in range(2):
            for cb in range(2):
                xp = xpads[q][cb]
                eng = nc.vector if q == 0 else nc.gpsimd
                eng.memset(xp[:, 0:60], 0.0)
                eng.memset(
                    xp[:, 116 : 116 + 55 * PH].rearrange(
                        "p (r w) -> p r w", w=PH
                    )[:, :, 0:2],
                    0.0,
                )
                eng.memset(xp[:, 3306:XLEN], 0.0)

        for n in range(NPER):
            q = n % 2
            xp0, xp1 = xpads[q]

            # ---- X^T via the DMA-transpose xbar, straight from DRAM.
            # Two batched transposes (12 k-chunks each) so t1 can start
            # after the first lands; the w=64 tail via PE. ----
            xt = xtp.tile([KP, KT * C], bf16, tag="xt", name="xt")
            for h in range(2):
                nc.scalar.dma_start(
                    out=xt[:, h * 12 * C : (h + 1) * 12 * C].rearrange(
                        "p (k c) -> p k c", c=C
                    ),
                    in_=x_d[n, :, h * 12 * KP : (h + 1) * 12 * KP],
                    transpose=True,
                )

            # ---- load x contiguously, both c-blocks in one DMA ----
            xraw = xrawp.tile([128, 2 * HW], bf16, tag="xraw", name="xraw")
            nc.sync.dma_start(
                out=xraw.rearrange("p (cb s) -> p cb s", cb=2),
                in_=x_d[n].rearrange("(cb p) s -> p cb s", cb=2),
            )

            # ---- tail transpose chunk (w=64) on the PE ----
            trp = ps_tail.tile([KP, 256], bf16, tag="tr", name="trp")
            for cb in range(2):
                nc.tensor.transpose(
                    trp[0:SLAST, cb * 128 : cb * 128 + 128],
                    xraw[:, cb * HW + KDMA * KP : cb * HW + HW],
                    ident[:, :],
                )
            nc.vector.tensor_copy(
                xt[0:SLAST, KDMA * C : KT * C], trp[0:SLAST, :]
            )

            # ---- build padded interior on-chip, on DVE (bf16 SBUF->SBUF
            # copies run 4x there; HW GpSimd took ~10.6us each) ----
            for cb, xp in ((0, xp0), (1, xp1)):
                dst = xp[:, 60 : 60 + 56 * PH].rearrange(
                    "p (r w) -> p r w", w=PH
                )[:, :, 0:56]
                src = xraw[:, cb * HW : (cb + 1) * HW].rearrange(
                    "p (r w) -> p r w", w=W
                )
                nc.vector.tensor_copy(dst, src)

            def conv_g(g, xp, c, pool):
                r0 = 1 + 8 * c
                # stream only the 448 real output columns per tap: the rhs
                # is a 2-free-dim AP [8 rows x 56 cols] with row stride 58,
                # skipping the 16 dead pad columns of the 464-wide window
                cv = ps_cv.tile([128, 448], f32, tag="cv", name="cv")
                for tap in range(9):
                    kh, kw = tap // 3, tap % 3
                    foff = (r0 + kh - 1) * PH + kw
                    nc.tensor.matmul(
                        cv[:, :],
                        wt_sb[
                            :, (g * 9 + tap) * 128 : (g * 9 + tap) * 128 + 128
                        ],
                        xp[:, foff + 1 : foff + 1 + 8 * PH].rearrange(
                            "p (r w) -> p r w", w=PH
                        )[:, :, 0:56],
                        start=(tap == 0),
                        stop=(tap == 8),
                    )
                t3g = pool.tile([128, 448], bf16, tag=f"t3{g}", name="t3g")
                nc.scalar.copy(t3g[:, :], cv[:, :])
                return t3g

            # ---- t1T = P^T @ X^T;  t4T = p4s * t1T  (b-part, a-free) ----
            t4T = svp.tile([128, 2 * C], bf16, tag="t4", name="t4T")
            for eb in range(2):
                t1ps = ps_acc.tile([128, C], f32, tag="acc", name="t1ps")
                for k in range(KT):
                    w = KP if k < KT - 1 else SLAST
                    nc.tensor.matmul(
                        t1ps[:, :],
                        p1_sb[0:w, k * C + eb * 128 : k * C + eb * 128 + 128],
                        xt[0:w, k * C : (k + 1) * C],
                        start=(k == 0),
                        stop=(k == KT - 1),
                    )
                nc.vector.tensor_mul(
                    t4T[:, eb * C : (eb + 1) * C],
                    t1ps[:, :],
                    p4_sb[:, eb * C : (eb + 1) * C],
                )

            # ---- t5[a] = sum_b t4T[b,a] p5[b] ----
            t5ps = ps_acc.tile([128, 2], f32, tag="acc", name="t5ps")
            for ab in range(2):
                for bb in range(2):
                    nc.tensor.matmul(
                        t5ps[:, ab : ab + 1],
                        t4T[:, bb * C + ab * 128 : bb * C + ab * 128 + 128],
                        p5_sb[:, bb : bb + 1],
                        start=(bb == 0),
                        stop=(bb == 1),
                    )
            t5col = svp.tile([128, 2], f32, tag="t5", name="t5col")
            nc.scalar.copy(t5col[:, :], t5ps[:, :])
            # t5 broadcast as a stationary matrix: t5b[cb][b, a] = t5[cb*128+b]
            # for every a -- the t7 row-broadcast then rides the PE.
            t5b = svp.tile([128, 2 * 128], bf16, tag="t5b", name="t5b")
            for cb in range(2):
                nc.vector.tensor_scalar_mul(
                    t5b[:, cb * 128 : (cb + 1) * 128],
                    ones_sb[:, :],
                    t5col[:, cb : cb + 1],
                )

            # ---- chunk loop: conv then t6(+t7), accumulate into obig ----
            ob = outp.tile([128, 2 * HW], bf16, tag="ob", name="ob")
            for c in range(NCHUNK):
                t3c = [conv_g(0, xp0, c, t3p), conv_g(1, xp1, c, t3p)]

                # broadcast t7 row for this chunk: every output partition
                # of t7ps gets t7[s] (t5b columns are all equal)
                t7ps = ps_t7.tile([128, 448], f32, tag="t7", name="t7ps")
                for cb in range(2):
                    nc.tensor.matmul(
                        t7ps[:, :],
                        t5b[:, cb * 128 : (cb + 1) * 128],
                        xraw[:, cb * HW + c * 448 : cb * HW + (c + 1) * 448],
                        start=(cb == 0),
                        stop=(cb == 1),
                    )
                t7b = svp.tile([128, 448], f32, tag="t7b", name="t7b")
                nc.scalar.copy(t7b[:, :], t7ps[:, :])

                for ab in range(2):
                    t6ps = ps_t6.tile([128, 448], f32, tag="t6", name="t6ps")
                    for bb in range(2):
                        nc.tensor.matmul(
                            t6ps[:, :],
                            t4T[:, bb * C + ab * 128 : bb * C + ab * 128 + 128],
                            t3c[bb][:, :],
                            start=(bb == 0),
                            stop=(bb == 1),
                        )
                    nc.vector.tensor_add(
                        ob[:, ab * HW + c * 448 : ab * HW + (c + 1) * 448],
                        t6ps[:, :],
                        t7b[:, :],
                    )

                # stage the store so only the last chunk's ~0.2MB of store
                # remains after the final add
                if c in (3, 5):
                    lo = 0 if c == 3 else 4 * 448
                    hi = (c + 1) * 448
                    nc.sync.dma_start(
                        out=out_d[n, :, lo:hi].rearrange(
                            "(ab p) s -> p ab s", ab=2
                        ),
                        in_=ob.rearrange("p (ab s) -> p ab s", ab=2)[
                            :, :, lo:hi
                        ],
                    )
            nc.sync.dma_start(
                out=out_d[n, :, 6 * 448 : HW].rearrange(
                    "(ab p) s -> p ab s", ab=2
                ),
                in_=ob.rearrange("p (ab s) -> p ab s", ab=2)[:, :, 6 * 448 : HW],
            )


_CACHE = {}


def _get_nc():
    if "nc" in _CACHE:
        return _CACHE["nc"]
    import concourse.bacc as bacc
    import concourse.mybir as mybir
    import concourse.tile as tile

    f32 = mybir.dt.float32
    bf16 = mybir.dt.bfloat16
    nc = bacc.Bacc(
        "TRN2", target_bir_lowering=False, debug=False, num_devices=NCORES
    )
    ins = {
        "x": nc.dram_tensor("x", (NPER, C, HW), bf16, kind="ExternalInput").ap(),
        "p1": nc.dram_tensor("p1", (KP, KT * C), bf16, kind="ExternalInput").ap(),
        "wt": nc.dram_tensor("wt", (128, 2 * 9 * 128), bf16, kind="ExternalInput").ap(),
        "p4s": nc.dram_tensor("p4s", (128, 2 * C), f32, kind="ExternalInput").ap(),
        "p5": nc.dram_tensor("p5", (128, 2), bf16, kind="ExternalInput").ap(),
        "ident": nc.dram_tensor("ident", (128, 128), bf16, kind="ExternalInput").ap(),
    }
    outs = {
        "out": nc.dram_tensor("out", (NPER, C, HW), bf16, kind="ExternalOutput").ap(),
    }
    with tile.TileContext(nc) as tc:
        build_body(tc, outs, ins)
    nc.compile()
    _CACHE["nc"] = nc
    return nc


def host_prep(inputs):
    """Split full inputs into per-core in_maps (host-side relayout + bf16)."""
    import ml_dtypes

    bf = ml_dtypes.bfloat16
    x = np.asarray(inputs["x"], dtype=np.float32).reshape(N, C, HW).astype(bf)
    p1p = np.zeros((KT * KP, C), dtype=np.float32)
    p1p[:HW] = np.asarray(inputs["p1_w"], dtype=np.float32)[..., 0].reshape(
        HW, C
    )
    p1h = np.ascontiguousarray(
        p1p.reshape(KT, KP, C).transpose(1, 0, 2).reshape(KP, KT * C)
    ).astype(bf)
    wt = np.ascontiguousarray(
        np.asarray(inputs["conv_w"], dtype=np.float32)
        .reshape(2, 128, 128, 9)
        .transpose(3, 0, 1, 2)       # t, g, o, i -> want [i, (g,t,o)]
        .transpose(3, 1, 0, 2)       # i, g, t, o
        .reshape(128, 2 * 9 * 128)
    ).astype(bf)
    p4s = np.ascontiguousarray(
        (np.asarray(inputs["p4_w"], dtype=np.float32)[0].T * INV)
        .reshape(2, 128, C)
        .transpose(1, 0, 2)
        .reshape(128, 2 * C)
    )
    identm = np.eye(128, dtype=np.float32).astype(bf)
    p5 = np.ascontiguousarray(
        np.asarray(inputs["p5_w"], dtype=np.float32).reshape(2, 128).T
    ).astype(bf)
    xs = x.reshape(NCORES, NPER, C, HW)
    return [
        {
            "x": np.ascontiguousarray(xs[i]), "p1": p1h, "wt": wt,
            "p4s": p4s, "p5": p5, "ident": identm,
        }
        for i in range(NCORES)
    ]


def kernel(**inputs):
    from concourse.bass_utils import run_bass_kernel_spmd

    nc = _get_nc()
    in_maps = host_prep(inputs)
    res = run_bass_kernel_spmd(nc, in_maps, core_ids=list(range(NCORES)))
    out = np.concatenate([res.results[i]["out"] for i in range(NCORES)], axis=0)
    return out.astype(np.float32).reshape(N, C, H, W)
